# revision 5
# baseline (speedup 1.0000x reference)
"""GQA attention forward (B=1, T=2048, DIM=2048, H=16, KV=4, HD=128) on 8 trn2 cores.

Sharding: tensor-parallel over heads. Core c owns q-heads {2c, 2c+1} and kv-head
c//2 (kv work duplicated across the pair of cores sharing it).

v2 schedule (vs v1): x/wq/wk/wv shipped+consumed as bf16 (halves input DMA);
phase-2 emits scores(u), AV(u-1) and wo matmuls round-robin at k-block
granularity so the PE never idles waiting on the ACT exp drain (keeps the PE
p-state ramped); softmax denominators accumulated on gpsimd (tensor_add of P
tiles) with a single ones-matmul partition-reduce per unit instead of a full
ones-matmul pass; reciprocal via the fast DVE approximation; wo output blocks
stream as soon as both heads of a q-chunk are normalized, overlapping the
output DMA with the remaining attention compute.

Per core: qT/kT/vT projections in [hd, t] layout (bf16 matmuls, f32 PSUM),
RoPE on-chip (partition-swap via SBUF-SBUF DMA + sign-folded sin table), v
PE-transposed to natural [t, hd] layout; scores S^T[k, q] = kT-block @ qT over
hd, exp on ACT with 1/sqrt(hd) folded into the activation scale, causal mask
via affine_select on the masked prefix only; A^T[hd, q] accumulates
V-block.T-contract @ P^T over k-blocks in PSUM; partial out = A^T.T @ woT_c.
Host: pre-transposes x/weights (x/w as bf16), sums the 8 partial [T, DIM]
outputs.
"""

import sys

if "/opt/trn_rl_repo" not in sys.path:
    sys.path.insert(0, "/opt/trn_rl_repo")

import numpy as np

T = 2048
DIM = 2048
H = 16
KV = 4
HD = 128
NCORES = 8
HPC = H // NCORES            # q heads per core = 2
SCALE = float(HD) ** -0.5
ND = DIM // 128              # dim chunks = 16
NT = T // 128                # t blocks = 16
NQC = T // 512               # q 512-chunks = 4

_CACHE = {}


def _build_nc():
    from contextlib import ExitStack

    from concourse import bacc
    import concourse.mybir as mybir
    import concourse.tile as tile
    from concourse.masks import make_identity

    f32 = mybir.dt.float32
    f32r = mybir.dt.float32r
    bf16 = mybir.dt.bfloat16
    Exp = mybir.ActivationFunctionType.Exp

    def r(ap):
        return ap.bitcast(f32r)

    def f(ap):
        return ap.bitcast(f32)

    nc = bacc.Bacc("TRN2", target_bir_lowering=False, debug=False,
                   enable_asserts=False)

    xT = nc.dram_tensor("xT", [DIM, T], bf16, kind="ExternalInput").ap()
    wqT = nc.dram_tensor("wqT", [DIM, HPC * HD], bf16, kind="ExternalInput").ap()
    wkT = nc.dram_tensor("wkT", [DIM, HD], bf16, kind="ExternalInput").ap()
    wvT = nc.dram_tensor("wvT", [DIM, HD], bf16, kind="ExternalInput").ap()
    woT = nc.dram_tensor("woT", [HPC * HD, DIM], f32r, kind="ExternalInput").ap()
    cosT = nc.dram_tensor("cosT", [HD, T], f32, kind="ExternalInput").ap()
    sinT = nc.dram_tensor("sinT", [HD, T], f32, kind="ExternalInput").ap()
    out = nc.dram_tensor("out", [T, DIM], f32, kind="ExternalOutput").ap()

    with tile.TileContext(nc) as tc, ExitStack() as ctx:
        const = ctx.enter_context(tc.tile_pool(name="const", bufs=1))
        wpool = ctx.enter_context(tc.tile_pool(name="wts", bufs=1))
        qkv = ctx.enter_context(tc.tile_pool(name="qkv", bufs=1))

        ident = const.tile([128, 128], f32)
        make_identity(nc, ident)
        ones_f = const.tile([128, 128], f32)
        nc.vector.memset(ones_f, 1.0)
        ones_s = const.tile([128, 128], f32r)
        nc.scalar.copy(ones_s, ones_f)

        qT_s = qkv.tile([128, HPC * T], f32r)
        kT_s = qkv.tile([128, T], f32r)
        vT_s = qkv.tile([128, T], f32)
        v_s = qkv.tile([128, NT * HD], f32r)   # natural [t%128, hd] per t-block

        # ---- Phase 1: projections + RoPE + v-transpose, four t-quarters ----
        with tc.tile_pool(name="xp", bufs=18) as xpool, \
             tc.tile_pool(name="rope", bufs=4) as rp, \
             tc.tile_pool(name="vtp", bufs=3, space="PSUM") as vtp, \
             tc.tile_pool(name="p1ps", bufs=3, space="PSUM") as ps1:

            def load_x_quarter(tq):
                xts = []
                for d in range(ND):
                    xt = xpool.tile([128, 512], bf16, tag="xt",
                                    name=f"xt{tq}_{d}")
                    nc.sync.dma_start(
                        xt, xT[d * 128:(d + 1) * 128,
                               tq * 512:(tq + 1) * 512])
                    xts.append(xt)
                return xts

            wk_s = wpool.tile([128, ND, HD], bf16)
            nc.sync.dma_start(wk_s, wkT.rearrange("(d p) n -> p d n", p=128))
            xq = [load_x_quarter(0)]
            wq_s = wpool.tile([128, ND, HPC * HD], bf16)
            nc.sync.dma_start(wq_s, wqT.rearrange("(d p) n -> p d n", p=128))
            wv_s = wpool.tile([128, ND, HD], bf16)
            nc.sync.dma_start(wv_s, wvT.rearrange("(d p) n -> p d n", p=128))
            cos_s = const.tile([128, T], f32)
            nc.sync.dma_start(cos_s, cosT)
            sin_s = const.tile([128, T], f32)
            nc.sync.dma_start(sin_s, sinT)
            xq.append(load_x_quarter(1))
            wo_s = wpool.tile([128, HPC, DIM], f32r)

            def rope(u, c0, t0, cols=512):
                us = u[:, c0:c0 + cols]
                rot = rp.tile([128, cols], f32r, tag="rot")
                nc.sync.dma_start(rot[0:64, :], us[64:128, :])
                nc.sync.dma_start(rot[64:128, :], us[0:64, :])
                tmp = rp.tile([128, cols], f32, tag="rtmp")
                nc.vector.tensor_mul(tmp, us, cos_s[:, t0:t0 + cols])
                nc.vector.tensor_mul(rot, rot, sin_s[:, t0:t0 + cols])
                nc.vector.tensor_add(us, tmp, rot)

            def proj(acc_tag, w_ap, xts, dst, c0):
                acc = ps1.tile([128, 512], f32, tag="pps", name=acc_tag)
                for d in range(ND):
                    nc.tensor.matmul(acc, w_ap(d), xts[d],
                                     start=(d == 0), stop=(d == ND - 1))
                nc.scalar.copy(dst[:, c0:c0 + 512], acc)

            for tq in range(4):
                if tq + 2 <= 3:
                    xq.append(load_x_quarter(tq + 2))
                if tq == 2:
                    # all x quarters issued; wo fills the now-idle DMA queues
                    nc.sync.dma_start(
                        wo_s, woT.rearrange("(h p) n -> p h n", p=128))
                xts = xq[tq]
                t0 = tq * 512
                proj("k", lambda d: wk_s[:, d, :], xts, kT_s, t0)
                rope(kT_s, t0, t0)
                for h in range(HPC):
                    proj(f"q{h}",
                         lambda d, h=h: wq_s[:, d, h * HD:(h + 1) * HD],
                         xts, qT_s, h * T + t0)
                    rope(qT_s, h * T + t0, t0)
                proj("v", lambda d: wv_s[:, d, :], xts, vT_s, t0)
                for tb in range(tq * 4, tq * 4 + 4):
                    vt = vtp.tile([128, 128], f32, tag="vt")
                    nc.tensor.transpose(
                        vt, vT_s[:, tb * 128:(tb + 1) * 128], ident)
                    nc.scalar.copy(
                        v_s[:, tb * HD:(tb + 1) * HD], vt)

        # ---- Phase 2+3: attention with fine-grained S/AV/wo interleave ----
        apool = ctx.enter_context(tc.tile_pool(name="Apool", bufs=1))
        aT_s = [apool.tile([128, T], f32r, name=f"aT{h}") for h in range(HPC)]
        units = [(qc, h) for qc in range(NQC) for h in range(HPC)]
        nkb_of = [4 * qc + 4 for (qc, h) in units]

        with tc.tile_pool(name="sps", bufs=3, space="PSUM") as sps, \
             tc.tile_pool(name="otp", bufs=2, space="PSUM") as otp, \
             tc.tile_pool(name="dnp", bufs=1, space="PSUM") as dnp, \
             tc.tile_pool(name="wops", bufs=2, space="PSUM") as wops, \
             tc.tile_pool(name="pp", bufs=24) as ppool, \
             tc.tile_pool(name="pacc", bufs=2) as paccp, \
             tc.tile_pool(name="rcp", bufs=2) as rpool, \
             tc.tile_pool(name="ost", bufs=8) as ostage:

            ptiles = {}     # u -> list of P tiles
            pacc = {}       # u -> accumulated P (f32)
            otiles = {}     # u -> oT PSUM tile

            def emit_S(u, kb):
                qc, h = units[u]
                qTh = qT_s[:, h * T:(h + 1) * T]
                s_ps = sps.tile([128, 512], f32, tag="s", name=f"s{u}_{kb}")
                nc.tensor.matmul(
                    s_ps, r(kT_s[:, kb * 128:(kb + 1) * 128]),
                    r(qTh[:, qc * 512:(qc + 1) * 512]),
                    start=True, stop=True)
                p_sb = ppool.tile([128, 512], f32r, tag="p",
                                  name=f"p{u}_{kb}")
                nc.scalar.activation(p_sb, s_ps, Exp, scale=SCALE)
                if kb >= 4 * qc:
                    # masked prefix: q_local < (kb*128 - qc*512) + k_local
                    w = (kb - 4 * qc) * 128 + 128
                    nc.gpsimd.affine_select(
                        out=p_sb[:, 0:w], in_=p_sb[:, 0:w],
                        compare_op=mybir.AluOpType.is_ge,
                        fill=0.0, base=qc * 512 - kb * 128,
                        channel_multiplier=-1, pattern=[[1, w]])
                if kb == 0:
                    pa = paccp.tile([128, 512], f32r, tag="pa",
                                    name=f"pa{u}")
                    pacc[u] = pa
                    nc.gpsimd.tensor_copy(pa, p_sb)
                else:
                    nc.gpsimd.tensor_add(pacc[u], pacc[u], p_sb)
                ptiles.setdefault(u, []).append(p_sb)

            def emit_AV(u, kb):
                nkb = nkb_of[u]
                if kb == 0:
                    otiles[u] = otp.tile([128, 512], f32, tag="oT",
                                         name=f"oT{u}")
                nc.tensor.matmul(
                    otiles[u], r(v_s[:, kb * HD:(kb + 1) * HD]),
                    r(ptiles[u][kb]),
                    start=(kb == 0), stop=(kb == nkb - 1))

            def finish_unit(u):
                qc, h = units[u]
                dn = dnp.tile([128, 512], f32, tag="dn", name=f"dn{u}")
                nc.tensor.matmul(dn, r(ones_s), r(pacc[u]),
                                 start=True, stop=True)
                rec = rpool.tile([128, 512], f32, tag="rec")
                nc.vector.reciprocal_approx_fast(rec, dn)
                nc.vector.tensor_mul(
                    aT_s[h][:, qc * 512:(qc + 1) * 512], otiles[u], rec)

            wo_queue = []

            def emit_wo_one():
                tb, n4 = wo_queue.pop(0)
                op = wops.tile([128, 512], f32, tag="op",
                               name=f"op{tb}_{n4}")
                for h in range(HPC):
                    nc.tensor.matmul(
                        op, r(aT_s[h][:, tb * 128:(tb + 1) * 128]),
                        r(wo_s[:, h, n4 * 512:(n4 + 1) * 512]),
                        start=(h == 0), stop=(h == HPC - 1))
                ob = ostage.tile([128, 512], f32, tag="ob")
                nc.vector.tensor_copy(ob, op)
                nc.sync.dma_start(
                    out[tb * 128:(tb + 1) * 128,
                        n4 * 512:(n4 + 1) * 512], ob)

            prev = None
            for u in range(len(units)):
                n_s = nkb_of[u]
                n_av = nkb_of[prev] if prev is not None else 0
                for i in range(max(n_s, n_av)):
                    if i < n_s:
                        emit_S(u, i)
                    if i < n_av:
                        emit_AV(prev, i)
                    if wo_queue:
                        emit_wo_one()
                if prev is not None:
                    finish_unit(prev)
                    qc_p, h_p = units[prev]
                    if h_p == HPC - 1:
                        wo_queue.extend(
                            (tb, n4)
                            for tb in range(qc_p * 4, qc_p * 4 + 4)
                            for n4 in range(4))
                prev = u
            for i in range(nkb_of[prev]):
                emit_AV(prev, i)
                if wo_queue:
                    emit_wo_one()
            finish_unit(prev)
            wo_queue.extend(
                (tb, n4) for tb in range(12, 16) for n4 in range(4))
            while wo_queue:
                emit_wo_one()

    nc.compile()
    return nc


def _shard_inputs(x, wq, wk, wv, wo, cos, sin):
    import ml_dtypes

    bf16 = ml_dtypes.bfloat16
    xTh = np.ascontiguousarray(x.reshape(T, DIM).T).astype(bf16)
    cosTh = np.ascontiguousarray(cos.T)
    # rotate_half sign fold: out = u*cos + u_rot*sin_signed
    sinTh = np.ascontiguousarray(sin.T).copy()
    sinTh[: HD // 2, :] *= -1.0
    in_maps = []
    for c in range(NCORES):
        g = c // 2
        in_maps.append({
            "xT": xTh,
            "wqT": np.ascontiguousarray(
                wq[c * HPC * HD:(c + 1) * HPC * HD, :].T).astype(bf16),
            "wkT": np.ascontiguousarray(
                wk[g * HD:(g + 1) * HD, :].T).astype(bf16),
            "wvT": np.ascontiguousarray(
                wv[g * HD:(g + 1) * HD, :].T).astype(bf16),
            "woT": np.ascontiguousarray(
                wo[:, c * HPC * HD:(c + 1) * HPC * HD].T),
            "cosT": cosTh,
            "sinT": sinTh,
        })
    return in_maps


def _get_exec():
    """Build (once) a cached jitted SPMD executable over the 8 cores.

    Mirrors bass2jax.run_bass_via_pjrt's multi-core branch, but caches the
    jitted callable so repeat kernel() calls don't re-trace/re-lower.
    """
    if "exec" in _CACHE:
        return _CACHE["exec"]

    import jax
    from jax.sharding import Mesh, PartitionSpec
    from jax.experimental.shard_map import shard_map
    from concourse import bass2jax
    import concourse.mybir as mybir

    if "nc" not in _CACHE:
        _CACHE["nc"] = _build_nc()
    nc = _CACHE["nc"]

    bass2jax.install_neuronx_cc_hook()

    part_name = (nc.partition_id_tensor.name
                 if nc.partition_id_tensor else None)
    in_names, out_names, out_avals = [], [], []
    for alloc in nc.m.functions[0].allocations:
        if not isinstance(alloc, mybir.MemoryLocationSet):
            continue
        name = alloc.memorylocations[0].name
        if alloc.kind == "ExternalInput":
            if name != part_name:
                in_names.append(name)
        elif alloc.kind == "ExternalOutput":
            out_names.append(name)
            out_avals.append(jax.core.ShapedArray(
                tuple(alloc.tensor_shape), mybir.dt.np(alloc.dtype)))

    bind_names = in_names + out_names
    if part_name is not None:
        bind_names = bind_names + [part_name]

    def _body(*args):
        operands = list(args)
        if part_name is not None:
            operands.append(bass2jax.partition_id_tensor())
        outs = bass2jax._bass_exec_p.bind(
            *operands,
            out_avals=tuple(out_avals),
            in_names=tuple(bind_names),
            out_names=tuple(out_names),
            lowering_input_output_aliases=(),
            sim_require_finite=True,
            sim_require_nnan=True,
            nc=nc,
        )
        return tuple(outs)

    devices = jax.devices()[:NCORES]
    mesh = Mesh(np.asarray(devices), ("core",))
    n_in = len(in_names)
    n_out = len(out_names)
    sharded = jax.jit(
        shard_map(
            _body, mesh=mesh,
            in_specs=(PartitionSpec("core"),) * (n_in + n_out),
            out_specs=(PartitionSpec("core"),) * n_out,
            check_rep=False,
        ),
        donate_argnums=tuple(range(n_in, n_in + n_out)),
        keep_unused=True,
    )
    _CACHE["body"] = _body
    _CACHE["exec"] = (sharded, in_names, out_names, out_avals, mesh)
    return _CACHE["exec"]


def _concat_inputs(in_maps, in_names):
    return [
        np.concatenate([in_maps[c][name] for c in range(NCORES)], axis=0)
        for name in in_names
    ]


def _zero_outs(out_avals):
    return [
        np.zeros((NCORES * a.shape[0], *a.shape[1:]), a.dtype)
        for a in out_avals
    ]


def kernel(**inputs):
    sharded, in_names, out_names, out_avals, _ = _get_exec()

    in_maps = _shard_inputs(
        np.asarray(inputs["x"], dtype=np.float32),
        np.asarray(inputs["wq"], dtype=np.float32),
        np.asarray(inputs["wk"], dtype=np.float32),
        np.asarray(inputs["wv"], dtype=np.float32),
        np.asarray(inputs["wo"], dtype=np.float32),
        np.asarray(inputs["cos"], dtype=np.float32),
        np.asarray(inputs["sin"], dtype=np.float32),
    )
    concat_in = _concat_inputs(in_maps, in_names)
    out_arrs = sharded(*concat_in, *_zero_outs(out_avals))

    full = np.asarray(out_arrs[out_names.index("out")])
    acc = full.reshape(NCORES, T, DIM).astype(np.float32).sum(axis=0)
    return acc.reshape(1, T, DIM)


# revision 14
# speedup vs baseline: 1.2484x; 1.2484x over previous
"""GQA attention forward (B=1, T=2048, DIM=2048, H=16, KV=4, HD=128) on 8 trn2 cores.

Sharding: tensor-parallel over heads. Core c owns q-heads {2c, 2c+1} and kv-head
c//2 (kv work duplicated across the pair of cores sharing it).

v2 schedule (vs v1): x/wq/wk/wv shipped+consumed as bf16 (halves input DMA);
phase-2 emits scores(u), AV(u-1) and wo matmuls round-robin at k-block
granularity so the PE never idles waiting on the ACT exp drain (keeps the PE
p-state ramped); softmax denominators accumulated on gpsimd (tensor_add of P
tiles) with a single ones-matmul partition-reduce per unit instead of a full
ones-matmul pass; reciprocal via the fast DVE approximation; wo output blocks
stream as soon as both heads of a q-chunk are normalized, overlapping the
output DMA with the remaining attention compute.

Per core: qT/kT/vT projections in [hd, t] layout (bf16 matmuls, f32 PSUM),
RoPE on-chip (partition-swap via SBUF-SBUF DMA + sign-folded sin table), v
PE-transposed to natural [t, hd] layout; scores S^T[k, q] = kT-block @ qT over
hd, exp on ACT with 1/sqrt(hd) folded into the activation scale, causal mask
via affine_select on the masked prefix only; A^T[hd, q] accumulates
V-block.T-contract @ P^T over k-blocks in PSUM; partial out = A^T.T @ woT_c.
Host: pre-transposes x/weights (x/w as bf16), sums the 8 partial [T, DIM]
outputs.
"""

import sys

if "/opt/trn_rl_repo" not in sys.path:
    sys.path.insert(0, "/opt/trn_rl_repo")

import numpy as np

T = 2048
DIM = 2048
H = 16
KV = 4
HD = 128
NCORES = 8
HPC = H // NCORES            # q heads per core = 2
SCALE = float(HD) ** -0.5
ND = DIM // 128              # dim chunks = 16
NT = T // 128                # t blocks = 16
NQC = T // 512               # q 512-chunks = 4

_CACHE = {}


def _build_nc():
    from contextlib import ExitStack

    from concourse import bacc
    import concourse.mybir as mybir
    import concourse.tile as tile
    from concourse.masks import make_identity

    f32 = mybir.dt.float32
    f32r = mybir.dt.float32r
    bf16 = mybir.dt.bfloat16
    Exp = mybir.ActivationFunctionType.Exp

    def r(ap):
        return ap.bitcast(f32r)

    def f(ap):
        return ap.bitcast(f32)

    nc = bacc.Bacc("TRN2", target_bir_lowering=False, debug=False,
                   enable_asserts=False)

    # weights host-prearranged to the on-chip layout so every DMA is a
    # plain contiguous copy (the strided rearrange DMAs cost ~6us at start)
    xT = nc.dram_tensor("xT", [DIM, T], bf16, kind="ExternalInput").ap()
    wqR = nc.dram_tensor("wqR", [128, ND, HPC * HD], bf16,
                         kind="ExternalInput").ap()
    wkR = nc.dram_tensor("wkR", [128, ND, HD], bf16,
                         kind="ExternalInput").ap()
    wvR = nc.dram_tensor("wvR", [128, ND, HD], bf16,
                         kind="ExternalInput").ap()
    woR = nc.dram_tensor("woR", [128, HPC, DIM], f32r,
                         kind="ExternalInput").ap()
    cosT = nc.dram_tensor("cosT", [HD, T], bf16, kind="ExternalInput").ap()
    sinT = nc.dram_tensor("sinT", [HD, T], bf16, kind="ExternalInput").ap()
    out = nc.dram_tensor("out", [T, DIM], f32, kind="ExternalOutput").ap()

    with tile.TileContext(nc) as tc, ExitStack() as ctx:
        const = ctx.enter_context(tc.tile_pool(name="const", bufs=1))
        wpool = ctx.enter_context(tc.tile_pool(name="wts", bufs=1))
        qkv = ctx.enter_context(tc.tile_pool(name="qkv", bufs=1))

        ident = const.tile([128, 128], f32)
        make_identity(nc, ident)
        ones_f = const.tile([128, 128], f32)
        nc.vector.memset(ones_f, 1.0)
        ones_s = const.tile([128, 128], f32r)
        nc.scalar.copy(ones_s, ones_f)

        qT_s = qkv.tile([128, HPC * T], f32r)
        kT_s = qkv.tile([128, T], f32r)
        vT_s = qkv.tile([128, T], f32)
        v_s = qkv.tile([128, NT * HD], f32r)   # natural [t%128, hd] per t-block

        # ---- Phase 1: projections + RoPE + v-transpose, four t-quarters ----
        with tc.tile_pool(name="xp", bufs=18) as xpool, \
             tc.tile_pool(name="rope", bufs=4) as rp, \
             tc.tile_pool(name="vtp", bufs=3, space="PSUM") as vtp, \
             tc.tile_pool(name="p1ps", bufs=3, space="PSUM") as ps1:

            def load_x_quarter(tq):
                xts = []
                for d in range(ND):
                    xt = xpool.tile([128, 512], bf16, tag="xt",
                                    name=f"xt{tq}_{d}")
                    nc.sync.dma_start(
                        xt, xT[d * 128:(d + 1) * 128,
                               tq * 512:(tq + 1) * 512])
                    xts.append(xt)
                return xts

            wk_s = wpool.tile([128, ND, HD], bf16)
            nc.sync.dma_start(wk_s, wkR)
            xq = [load_x_quarter(0)]
            cos_s = const.tile([128, T], bf16)
            nc.sync.dma_start(cos_s, cosT)
            sin_s = const.tile([128, T], bf16)
            nc.sync.dma_start(sin_s, sinT)
            wq_s = wpool.tile([128, ND, HPC * HD], bf16)
            nc.sync.dma_start(wq_s, wqR)
            wv_s = wpool.tile([128, ND, HD], bf16)
            nc.sync.dma_start(wv_s, wvR)
            xq.append(load_x_quarter(1))
            wo_s = wpool.tile([128, HPC, DIM], f32r)

            def rope(u, c0, t0, cols=512):
                us = u[:, c0:c0 + cols]
                rot = rp.tile([128, cols], f32r, tag="rot")
                nc.sync.dma_start(rot[0:64, :], us[64:128, :])
                nc.sync.dma_start(rot[64:128, :], us[0:64, :])
                tmp = rp.tile([128, cols], f32, tag="rtmp")
                nc.vector.tensor_mul(tmp, us, cos_s[:, t0:t0 + cols])
                nc.vector.tensor_mul(rot, rot, sin_s[:, t0:t0 + cols])
                nc.vector.tensor_add(us, tmp, rot)

            def proj(acc_tag, w_ap, xts, dst, c0):
                acc = ps1.tile([128, 512], f32, tag="pps", name=acc_tag)
                for d in range(ND):
                    nc.tensor.matmul(acc, w_ap(d), xts[d],
                                     start=(d == 0), stop=(d == ND - 1))
                nc.scalar.copy(dst[:, c0:c0 + 512], acc)

            for tq in range(4):
                if tq + 2 <= 3:
                    xq.append(load_x_quarter(tq + 2))
                if tq == 2:
                    # all x quarters issued; wo fills the now-idle DMA queues
                    nc.sync.dma_start(wo_s, woR)
                xts = xq[tq]
                t0 = tq * 512
                proj("k", lambda d: wk_s[:, d, :], xts, kT_s, t0)
                rope(kT_s, t0, t0)
                for h in range(HPC):
                    proj(f"q{h}",
                         lambda d, h=h: wq_s[:, d, h * HD:(h + 1) * HD],
                         xts, qT_s, h * T + t0)
                    rope(qT_s, h * T + t0, t0)
                proj("v", lambda d: wv_s[:, d, :], xts, vT_s, t0)
                for tb in range(tq * 4, tq * 4 + 4):
                    vt = vtp.tile([128, 128], f32, tag="vt")
                    nc.tensor.transpose(
                        vt, vT_s[:, tb * 128:(tb + 1) * 128], ident)
                    nc.scalar.copy(
                        v_s[:, tb * HD:(tb + 1) * HD], vt)

        # ---- Phase 2+3: attention with fine-grained S/AV/wo interleave ----
        apool = ctx.enter_context(tc.tile_pool(name="Apool", bufs=1))
        aT_s = [apool.tile([128, T], f32r, name=f"aT{h}") for h in range(HPC)]
        # descending qc: the largest softmax unit runs first so its long
        # denominator-accumulate chain never sits on the critical tail
        units = [(qc, h) for qc in range(NQC - 1, -1, -1) for h in range(HPC)]
        nkb_of = [4 * qc + 4 for (qc, h) in units]

        with tc.tile_pool(name="sps", bufs=3, space="PSUM") as sps, \
             tc.tile_pool(name="otp", bufs=2, space="PSUM") as otp, \
             tc.tile_pool(name="dnp", bufs=1, space="PSUM") as dnp, \
             tc.tile_pool(name="wops", bufs=2, space="PSUM") as wops, \
             tc.tile_pool(name="pp", bufs=26) as ppool, \
             tc.tile_pool(name="pacc", bufs=2) as paccp, \
             tc.tile_pool(name="rcp", bufs=2) as rpool, \
             tc.tile_pool(name="ost", bufs=8) as ostage:

            ptiles = {}     # u -> list of P tiles
            pacc = {}       # u -> accumulated P (f32)
            otiles = {}     # u -> oT PSUM tile

            def emit_S(u, kb):
                qc, h = units[u]
                qTh = qT_s[:, h * T:(h + 1) * T]
                s_ps = sps.tile([128, 512], f32, tag="s", name=f"s{u}_{kb}")
                nc.tensor.matmul(
                    s_ps, r(kT_s[:, kb * 128:(kb + 1) * 128]),
                    r(qTh[:, qc * 512:(qc + 1) * 512]),
                    start=True, stop=True)
                p_sb = ppool.tile([128, 512], f32r, tag="p",
                                  name=f"p{u}_{kb}")
                nc.scalar.activation(p_sb, s_ps, Exp, scale=SCALE)
                if kb >= 4 * qc:
                    # masked prefix: q_local < (kb*128 - qc*512) + k_local
                    w = (kb - 4 * qc) * 128 + 128
                    nc.gpsimd.affine_select(
                        out=p_sb[:, 0:w], in_=p_sb[:, 0:w],
                        compare_op=mybir.AluOpType.is_ge,
                        fill=0.0, base=qc * 512 - kb * 128,
                        channel_multiplier=-1, pattern=[[1, w]])
                ptiles.setdefault(u, []).append(p_sb)
                if kb == 1:
                    pa = paccp.tile([128, 512], f32r, tag="pa",
                                    name=f"pa{u}")
                    pacc[u] = pa
                    nc.vector.tensor_add(pa, ptiles[u][0], p_sb)
                elif kb > 1:
                    nc.vector.tensor_add(pacc[u], pacc[u], p_sb)

            def emit_AV(u, kb):
                nkb = nkb_of[u]
                if kb == 0:
                    otiles[u] = otp.tile([128, 512], f32, tag="oT",
                                         name=f"oT{u}")
                nc.tensor.matmul(
                    otiles[u], r(v_s[:, kb * HD:(kb + 1) * HD]),
                    r(ptiles[u][kb]),
                    start=(kb == 0), stop=(kb == nkb - 1))

            def finish_unit(u):
                qc, h = units[u]
                dn = dnp.tile([128, 512], f32, tag="dn", name=f"dn{u}")
                nc.tensor.matmul(dn, r(ones_s), r(pacc[u]),
                                 start=True, stop=True)
                rec = rpool.tile([128, 512], f32, tag="rec")
                nc.vector.reciprocal_approx_fast(rec, dn)
                nc.vector.tensor_mul(
                    aT_s[h][:, qc * 512:(qc + 1) * 512], otiles[u], rec)

            wo_queue = []

            def emit_wo_one():
                tb, n4 = wo_queue.pop(0)
                op = wops.tile([128, 512], f32, tag="op",
                               name=f"op{tb}_{n4}")
                for h in range(HPC):
                    nc.tensor.matmul(
                        op, r(aT_s[h][:, tb * 128:(tb + 1) * 128]),
                        r(wo_s[:, h, n4 * 512:(n4 + 1) * 512]),
                        start=(h == 0), stop=(h == HPC - 1))
                ob = ostage.tile([128, 512], f32, tag="ob")
                nc.scalar.copy(ob, op)
                nc.sync.dma_start(
                    out[tb * 128:(tb + 1) * 128,
                        n4 * 512:(n4 + 1) * 512], ob)

            prev = None
            for u in range(len(units)):
                n_s = nkb_of[u]
                n_av = nkb_of[prev] if prev is not None else 0
                for i in range(max(n_s, n_av)):
                    if i < n_s:
                        emit_S(u, i)
                    if i < n_av:
                        emit_AV(prev, i)
                    if wo_queue:
                        emit_wo_one()
                if prev is not None:
                    finish_unit(prev)
                    qc_p, h_p = units[prev]
                    if h_p == HPC - 1:
                        wo_queue.extend(
                            (tb, n4)
                            for tb in range(qc_p * 4, qc_p * 4 + 4)
                            for n4 in range(4))
                prev = u
            for i in range(nkb_of[prev]):
                emit_AV(prev, i)
                if wo_queue:
                    emit_wo_one()
            finish_unit(prev)
            qc_p = units[prev][0]
            wo_queue.extend(
                (tb, n4)
                for tb in range(qc_p * 4, qc_p * 4 + 4) for n4 in range(4))
            while wo_queue:
                emit_wo_one()

    nc.compile()
    return nc


def _shard_inputs(x, wq, wk, wv, wo, cos, sin):
    import ml_dtypes

    bf16 = ml_dtypes.bfloat16

    def rearr_w(wT, ncols):
        # [DIM, ncols] -> [128, ND, ncols]: chip layout, contiguous DMA
        return np.ascontiguousarray(
            wT.reshape(ND, 128, ncols).transpose(1, 0, 2))

    xTh = np.ascontiguousarray(x.reshape(T, DIM).T).astype(bf16)
    cosTh = np.ascontiguousarray(cos.T).astype(bf16)
    # rotate_half sign fold: out = u*cos + u_rot*sin_signed
    sinTh = np.ascontiguousarray(sin.T).copy()
    sinTh[: HD // 2, :] *= -1.0
    sinTh = sinTh.astype(bf16)
    in_maps = []
    for c in range(NCORES):
        g = c // 2
        woc = wo[:, c * HPC * HD:(c + 1) * HPC * HD].T  # [HPC*HD, DIM]
        in_maps.append({
            "xT": xTh,
            "wqR": rearr_w(
                wq[c * HPC * HD:(c + 1) * HPC * HD, :].T.astype(bf16),
                HPC * HD),
            "wkR": rearr_w(wk[g * HD:(g + 1) * HD, :].T.astype(bf16), HD),
            "wvR": rearr_w(wv[g * HD:(g + 1) * HD, :].T.astype(bf16), HD),
            "woR": np.ascontiguousarray(
                woc.reshape(HPC, 128, DIM).transpose(1, 0, 2)),
            "cosT": cosTh,
            "sinT": sinTh,
        })
    return in_maps


def _get_exec():
    """Build (once) a cached jitted SPMD executable over the 8 cores.

    Mirrors bass2jax.run_bass_via_pjrt's multi-core branch, but caches the
    jitted callable so repeat kernel() calls don't re-trace/re-lower.
    """
    if "exec" in _CACHE:
        return _CACHE["exec"]

    import jax
    from jax.sharding import Mesh, PartitionSpec
    from jax.experimental.shard_map import shard_map
    from concourse import bass2jax
    import concourse.mybir as mybir

    if "nc" not in _CACHE:
        _CACHE["nc"] = _build_nc()
    nc = _CACHE["nc"]

    bass2jax.install_neuronx_cc_hook()

    part_name = (nc.partition_id_tensor.name
                 if nc.partition_id_tensor else None)
    in_names, out_names, out_avals = [], [], []
    for alloc in nc.m.functions[0].allocations:
        if not isinstance(alloc, mybir.MemoryLocationSet):
            continue
        name = alloc.memorylocations[0].name
        if alloc.kind == "ExternalInput":
            if name != part_name:
                in_names.append(name)
        elif alloc.kind == "ExternalOutput":
            out_names.append(name)
            out_avals.append(jax.core.ShapedArray(
                tuple(alloc.tensor_shape), mybir.dt.np(alloc.dtype)))

    bind_names = in_names + out_names
    if part_name is not None:
        bind_names = bind_names + [part_name]

    def _body(*args):
        operands = list(args)
        if part_name is not None:
            operands.append(bass2jax.partition_id_tensor())
        outs = bass2jax._bass_exec_p.bind(
            *operands,
            out_avals=tuple(out_avals),
            in_names=tuple(bind_names),
            out_names=tuple(out_names),
            lowering_input_output_aliases=(),
            sim_require_finite=True,
            sim_require_nnan=True,
            nc=nc,
        )
        return tuple(outs)

    devices = jax.devices()[:NCORES]
    mesh = Mesh(np.asarray(devices), ("core",))
    n_in = len(in_names)
    n_out = len(out_names)
    sharded = jax.jit(
        shard_map(
            _body, mesh=mesh,
            in_specs=(PartitionSpec("core"),) * (n_in + n_out),
            out_specs=(PartitionSpec("core"),) * n_out,
            check_rep=False,
        ),
        donate_argnums=tuple(range(n_in, n_in + n_out)),
        keep_unused=True,
    )
    _CACHE["body"] = _body
    _CACHE["exec"] = (sharded, in_names, out_names, out_avals, mesh)
    return _CACHE["exec"]


def _concat_inputs(in_maps, in_names):
    return [
        np.concatenate([in_maps[c][name] for c in range(NCORES)], axis=0)
        for name in in_names
    ]


def _zero_outs(out_avals):
    return [
        np.zeros((NCORES * a.shape[0], *a.shape[1:]), a.dtype)
        for a in out_avals
    ]


def kernel(**inputs):
    sharded, in_names, out_names, out_avals, _ = _get_exec()

    in_maps = _shard_inputs(
        np.asarray(inputs["x"], dtype=np.float32),
        np.asarray(inputs["wq"], dtype=np.float32),
        np.asarray(inputs["wk"], dtype=np.float32),
        np.asarray(inputs["wv"], dtype=np.float32),
        np.asarray(inputs["wo"], dtype=np.float32),
        np.asarray(inputs["cos"], dtype=np.float32),
        np.asarray(inputs["sin"], dtype=np.float32),
    )
    concat_in = _concat_inputs(in_maps, in_names)
    out_arrs = sharded(*concat_in, *_zero_outs(out_avals))

    full = np.asarray(out_arrs[out_names.index("out")])
    acc = full.reshape(NCORES, T, DIM).astype(np.float32).sum(axis=0)
    return acc.reshape(1, T, DIM)
